# revision 10
# baseline (speedup 1.0000x reference)
"""Trainium2 Bass kernel for per-sample dynamic 3x3 conv (periodic padding).

y[b,o,h,w] = sum_{c,i,j} x[b,c,(h+i-1)%H,(w+j-1)%W] * wgt[b, c*9+i*3+j, o] + bias[b,o]

Shapes: x [16,64,128,128] f32, wgt [16,576,64] f32, bias [16,64] f32.

Sharding: data-parallel over batch, 2 samples per core on 8 cores.

Per-core compute scheme: both samples are packed into single 128x128
matmuls with block-diagonal stationary weights:
  lhsT[k,m] = W_s0[c,o] for (k=c, m=o), W_s1[c,o] for (k=64+c, m=64+o), else 0
  rhs[k,n]  = padded_img_s0[c, pos] (k<64) / padded_img_s1[c, pos] (k>=64)
so one matmul per 3x3 shift contracts C=64 for both samples at once
(full K=128, full M=128). 9 shifts accumulate into one PSUM bank per
spatial tile of 4 image rows (N=512). Matmuls run as float32r views
(1 cycle/row at N>=256 vs 4 for plain fp32).
"""

import numpy as np

KH = KW = 3
B, C, O, H, W = 16, 64, 64, 128, 128
N_CORES = 8
BPC = B // N_CORES  # samples per core
HP, WP = H + 2, W + 2  # 130x130 periodic-padded image
TILE_ROWS = 4  # output rows per PSUM tile -> N = 4*128 = 512
N_TILES = H // TILE_ROWS
LOAD_CHUNK = 16  # image rows per interior load DMA

_CACHE = {}


def _patch_tile_drain():
    """This container's walrus rejects Drain instructions carrying more than
    one sem wait (setupSyncWait: Too many sync wait commands). Re-emit the
    TileContext exit drain's waits as individual wait_ge instructions."""
    import concourse.tile as tile
    from concourse.vector_clock import ScopedClock

    if getattr(tile.TileContext, "_drain_patch_applied", False):
        return

    def _drain_and_barrier(self, tick_clock, wait_clock):
        nc = self.nc
        nop = nc.sync.nop(nofuse=True)
        wait_clock.add_sem_waits(nop.ins, ScopedClock({None: tick_clock.global_clock}))
        waits = list(nop.ins.sync_info.on_wait)
        nop.ins.sync_info.on_wait.clear()
        assert self.sems is not None
        by_name = {}
        for h in self.sems.allocated().values():
            by_name[getattr(h, "name", None)] = h
        for w in waits:
            h = by_name.get(w.ant_name)
            assert h is not None, f"no sem handle for {w.ant_name}"
            nc.sync.wait_ge(h, w.wait_value)
        nc.sync.drain()
        nc.all_engine_barrier()
        popped = nc._tile_sem_poison_stack.pop()
        assert popped is self._sem_poison
        nc.clear_and_free_semaphores(list(self.sems.allocated().values()))
        nc.all_engine_barrier()

    tile.TileContext._drain_and_barrier = _drain_and_barrier
    tile.TileContext._drain_patch_applied = True


def _split_multi_waits(nc, max_waits=1):
    """Same walrus limitation, general form: any instruction carrying more
    than one sem wait fails setupSyncWait. Hoist excess waits onto dedicated
    single-wait NOPs on the same engine, placed just before the instruction."""
    import concourse.mybir as mybir

    for f in nc.m.functions:
        for blk in f.blocks:
            out = []
            changed = False
            for inst in blk.instructions:
                si = getattr(inst, "sync_info", None)
                waits = list(si.on_wait) if si is not None else []
                if len(waits) > max_waits:
                    changed = True
                    for w in waits[:-max_waits]:
                        out.append(
                            mybir.InstNoOp(
                                name=nc.get_next_instruction_name(),
                                engine=inst.engine,
                                sync_info=mybir.SyncInfo(on_wait=[w], on_update=[]),
                                bass_nofuse=True,
                            )
                        )
                    si.on_wait.clear()
                    for w in waits[-max_waits:]:
                        si.on_wait.append(w)
                out.append(inst)
            if changed:
                blk.instructions = out


def _build_module():
    import concourse.bass as bass
    import concourse.mybir as mybir
    import concourse.tile as tile

    _patch_tile_drain()

    f32 = mybir.dt.float32
    f32r = mybir.dt.float32r

    nc = bass.Bass()
    # input/weight feed FP32r matmuls; the BIR verifier requires every
    # producer in that dataflow to be float32r-typed, so declare them (and
    # the SBUF tiles) as float32r end-to-end. float32r is byte-identical to
    # float32 on the host side (dt.np(float32r) == np.float32).
    x_d = nc.dram_tensor("input", [BPC, C, H, W], f32r, kind="ExternalInput")
    w_d = nc.dram_tensor("weight", [BPC, C * KH * KW, O], f32r, kind="ExternalInput")
    b_d = nc.dram_tensor("bias", [BPC, O], f32, kind="ExternalInput")
    # host-supplied zeros for the block-diag off-diagonal fill: memset can't
    # produce float32r (invalid ISA value type), a DMA can.
    z_d = nc.dram_tensor("zeros", [KH * KW * 128], f32r, kind="ExternalInput")
    y_d = nc.dram_tensor("out", [BPC, O, H, W], f32, kind="ExternalOutput")

    with tile.TileContext(nc) as tc:
        from contextlib import ExitStack

        ctx = ExitStack()
        with ctx:
            persist = ctx.enter_context(tc.tile_pool(name="persist", bufs=1))
            psum = ctx.enter_context(tc.tile_pool(name="psum", bufs=6, space="PSUM"))
            ostage = ctx.enter_context(tc.tile_pool(name="ostage", bufs=6))

            # --- weights: block-diag per shift, [128 parts, 9 shifts, 128] ---
            wts = persist.tile([128, KH * KW, 128], f32r)
            z_src = z_d.rearrange("(s o) -> s o", s=KH * KW)
            nc.sync.dma_start(
                out=wts,
                in_=bass.AP(
                    tensor=z_src.tensor,
                    offset=z_src.offset,
                    ap=[[0, 128], z_src.ap[0], z_src.ap[1]],
                ),
            )
            for b in range(BPC):
                w_s = w_d[b].rearrange("(c s) o -> s c o", s=KH * KW)  # [9,64,64]
                for s in range(KH * KW):
                    nc.sync.dma_start(
                        out=wts[64 * b : 64 * b + 64, s, 64 * b : 64 * b + 64],
                        in_=w_s[s],
                    )

            # --- bias: [128, 1], s0 channels on parts 0-63, s1 on 64-127 ---
            bias_sb = persist.tile([128, 1], f32)
            nc.sync.dma_start(
                out=bias_sb, in_=b_d.rearrange("b o -> (b o)").rearrange("(p x) -> p x", x=1)
            )

            # --- padded images: [128 parts, 130, 130]; parts 0-63 = s0 chans ---
            img = persist.tile([128, HP, WP], f32r)
            n_chunks = H // LOAD_CHUNK
            for b in range(BPC):
                p0 = 64 * b
                for k in range(n_chunks):
                    r0 = k * LOAD_CHUNK
                    nc.sync.dma_start(
                        out=img[p0 : p0 + 64, 1 + r0 : 1 + r0 + LOAD_CHUNK, 1 : 1 + W],
                        in_=x_d[b, :, r0 : r0 + LOAD_CHUNK, :],
                    )
                # periodic row halos: padded row 0 <- x row H-1, row 129 <- x row 0
                nc.sync.dma_start(
                    out=img[p0 : p0 + 64, 0, 1 : 1 + W], in_=x_d[b, :, H - 1, :]
                )
                nc.sync.dma_start(
                    out=img[p0 : p0 + 64, HP - 1, 1 : 1 + W], in_=x_d[b, :, 0, :]
                )
            # periodic col halos (both samples at once), chunked by rows so
            # compute on early tiles doesn't wait for the whole image:
            # col 0 <- col 128 (x col W-1), col 129 <- col 1 (x col 0)
            col_ranges = [(0, 1 + LOAD_CHUNK)]
            for k in range(1, n_chunks - 1):
                col_ranges.append((1 + k * LOAD_CHUNK, 1 + (k + 1) * LOAD_CHUNK))
            col_ranges.append((1 + (n_chunks - 1) * LOAD_CHUNK, HP))
            for r0, r1 in col_ranges:
                nc.vector.tensor_copy(
                    out=img[:, r0:r1, 0], in_=img[:, r0:r1, W]
                )
                nc.vector.tensor_copy(
                    out=img[:, r0:r1, WP - 1], in_=img[:, r0:r1, 1]
                )

            # --- main loop: 32 spatial tiles of 4 output rows ---
            for t in range(N_TILES):
                h0 = t * TILE_ROWS
                ps = psum.tile([128, TILE_ROWS, W], f32)
                s = 0
                for i in range(KH):
                    for j in range(KW):
                        rhs = img[:, h0 + i : h0 + i + TILE_ROWS, j : j + W]
                        nc.tensor.matmul(
                            ps[:, :, :],
                            lhsT=wts[:, s, :],
                            rhs=rhs,
                            start=(s == 0),
                            stop=(s == KH * KW - 1),
                        )
                        s += 1
                st = ostage.tile([128, TILE_ROWS, W], f32)
                nc.scalar.activation(
                    out=st, in_=ps, func=mybir.ActivationFunctionType.Identity,
                    bias=bias_sb,
                )
                for b in range(BPC):
                    nc.sync.dma_start(
                        out=y_d[b, :, h0 : h0 + TILE_ROWS, :],
                        in_=st[64 * b : 64 * b + 64],
                    )
    return nc


def _get_module():
    if "nc" not in _CACHE:
        nc = _build_module()
        # CoreSim can't run modules with post-inserted instructions, so the
        # wait split is applied only on the hardware path.
        _split_multi_waits(nc)
        _CACHE["nc"] = nc
    return _CACHE["nc"]


def _in_maps(input, weight, bias):
    zeros = np.zeros(KH * KW * 128, np.float32)
    maps = []
    for i in range(N_CORES):
        lo, hi = i * BPC, (i + 1) * BPC
        maps.append(
            {
                "input": np.ascontiguousarray(input[lo:hi]),
                "weight": np.ascontiguousarray(weight[lo:hi]),
                "bias": np.ascontiguousarray(bias[lo:hi]),
                "zeros": zeros,
            }
        )
    return maps


def kernel(input, weight, bias):
    from concourse.bass_utils import run_bass_kernel_spmd

    nc = _get_module()
    res = run_bass_kernel_spmd(
        nc, _in_maps(input, weight, bias), core_ids=list(range(N_CORES))
    )
    return np.concatenate([res.results[i]["out"] for i in range(N_CORES)], axis=0)


# revision 11
# speedup vs baseline: 1.1569x; 1.1569x over previous
"""Trainium2 Bass kernel for per-sample dynamic 3x3 conv (periodic padding).

y[b,o,h,w] = sum_{c,i,j} x[b,c,(h+i-1)%H,(w+j-1)%W] * wgt[b, c*9+i*3+j, o] + bias[b,o]

Shapes: x [16,64,128,128] f32, wgt [16,576,64] f32, bias [16,64] f32.

Sharding: data-parallel over batch, 2 samples per core on 8 cores.

Compute scheme: both per-core samples are packed into single 128x128
matmuls with block-diagonal stationary weights:
  lhsT[k,m] = W_s0[c,o] at (k=c, m=o), W_s1[c,o] at (k=64+c, m=64+o), else 0
  rhs[k,n]  = col-padded img_s0[c, pos] (k<64) / img_s1[c, pos] (k>=64)
so one matmul per 3x3 shift contracts C=64 for both samples at once
(full K=128, full M=128, N=512). Matmuls are float32r (1 cycle/row).

Data movement (descriptor-count bound on this part, so): images load
CONTIGUOUSLY into a raw SBUF tile (16KB runs); the column-wrap-padded
image [128, 128, 130] is built on-chip by DVE/ACT/GPSIMD copies; the
row wrap is handled by splitting the affected matmuls on the two
boundary spatial tiles. Output rows are staged 8 at a time so stores
are 4KB-contiguous-per-partition DMAs.
"""

import numpy as np

KH = KW = 3
B, C, O, H, W = 16, 64, 64, 128, 128
N_CORES = 8
BPC = B // N_CORES  # samples per core
WP = W + 2  # 130: column-wrap padded row length
TILE_ROWS = 4  # output rows per PSUM tile -> N = 4*128 = 512
N_TILES = H // TILE_ROWS
LOAD_CHUNK = 32  # image rows per interior load DMA / pad-build chunk

_CACHE = {}


def _patch_tile_drain():
    """This container's walrus rejects Drain instructions carrying more than
    one sem wait (setupSyncWait: Too many sync wait commands). Re-emit the
    TileContext exit drain's waits as individual wait_ge instructions."""
    import concourse.tile as tile
    from concourse.vector_clock import ScopedClock

    if getattr(tile.TileContext, "_drain_patch_applied", False):
        return

    def _drain_and_barrier(self, tick_clock, wait_clock):
        nc = self.nc
        nop = nc.sync.nop(nofuse=True)
        wait_clock.add_sem_waits(nop.ins, ScopedClock({None: tick_clock.global_clock}))
        waits = list(nop.ins.sync_info.on_wait)
        nop.ins.sync_info.on_wait.clear()
        assert self.sems is not None
        by_name = {}
        for h in self.sems.allocated().values():
            by_name[getattr(h, "name", None)] = h
        for w in waits:
            h = by_name.get(w.ant_name)
            assert h is not None, f"no sem handle for {w.ant_name}"
            nc.sync.wait_ge(h, w.wait_value)
        nc.sync.drain()
        nc.all_engine_barrier()
        popped = nc._tile_sem_poison_stack.pop()
        assert popped is self._sem_poison
        nc.clear_and_free_semaphores(list(self.sems.allocated().values()))
        nc.all_engine_barrier()

    tile.TileContext._drain_and_barrier = _drain_and_barrier
    tile.TileContext._drain_patch_applied = True


def _split_multi_waits(nc, max_waits=1):
    """Same walrus limitation, general form: any instruction carrying more
    than one sem wait fails setupSyncWait. Hoist excess waits onto dedicated
    single-wait NOPs on the same engine, placed just before the instruction."""
    import concourse.mybir as mybir

    for f in nc.m.functions:
        for blk in f.blocks:
            out = []
            changed = False
            for inst in blk.instructions:
                si = getattr(inst, "sync_info", None)
                waits = list(si.on_wait) if si is not None else []
                if len(waits) > max_waits:
                    changed = True
                    for w in waits[:-max_waits]:
                        out.append(
                            mybir.InstNoOp(
                                name=nc.get_next_instruction_name(),
                                engine=inst.engine,
                                sync_info=mybir.SyncInfo(on_wait=[w], on_update=[]),
                                bass_nofuse=True,
                            )
                        )
                    si.on_wait.clear()
                    for w in waits[-max_waits:]:
                        si.on_wait.append(w)
                out.append(inst)
            if changed:
                blk.instructions = out


def _build_module():
    import concourse.bass as bass
    import concourse.mybir as mybir
    import concourse.tile as tile

    _patch_tile_drain()

    f32 = mybir.dt.float32
    f32r = mybir.dt.float32r

    nc = bass.Bass()
    # input/weight feed FP32r matmuls; the BIR verifier requires every
    # producer in that dataflow to be float32r-typed, so declare the whole
    # chain float32r. float32r is byte-identical to float32 host-side.
    x_d = nc.dram_tensor("input", [BPC, C, H, W], f32r, kind="ExternalInput")
    w_d = nc.dram_tensor("weight", [BPC, C * KH * KW, O], f32r, kind="ExternalInput")
    b_d = nc.dram_tensor("bias", [BPC, O], f32, kind="ExternalInput")
    # host-supplied zeros for the block-diag off-diagonal fill: memset can't
    # produce float32r (invalid ISA value type), a DMA can.
    z_d = nc.dram_tensor("zeros", [KH * KW * 128], f32r, kind="ExternalInput")
    y_d = nc.dram_tensor("out", [BPC, O, H, W], f32, kind="ExternalOutput")

    with tile.TileContext(nc) as tc:
        from contextlib import ExitStack

        ctx = ExitStack()
        with ctx:
            persist = ctx.enter_context(tc.tile_pool(name="persist", bufs=1))
            psum = ctx.enter_context(tc.tile_pool(name="psum", bufs=6, space="PSUM"))
            ostage = ctx.enter_context(tc.tile_pool(name="ostage", bufs=4))

            # --- weights: block-diag per shift, [128 parts, 9 shifts, 128].
            # Off-diag zero fill via broadcast DMA, then one DMA per sample:
            # per partition c the source rows c*9+s (s=0..8) are 9*64
            # contiguous floats.
            wts = persist.tile([128, KH * KW, 128], f32r)
            z_src = z_d.rearrange("(s o) -> s o", s=KH * KW)
            nc.sync.dma_start(
                out=wts,
                in_=bass.AP(
                    tensor=z_src.tensor,
                    offset=z_src.offset,
                    ap=[[0, 128], z_src.ap[0], z_src.ap[1]],
                ),
            )
            for b in range(BPC):
                w_cso = w_d[b].rearrange("(c s) o -> c s o", s=KH * KW)  # [64,9,64]
                nc.sync.dma_start(
                    out=wts[64 * b : 64 * b + 64, :, 64 * b : 64 * b + 64],
                    in_=w_cso,
                )

            # --- bias: [128, 1], s0 channels on parts 0-63, s1 on 64-127 ---
            bias_sb = persist.tile([128, 1], f32)
            nc.sync.dma_start(
                out=bias_sb,
                in_=b_d.rearrange("b o -> (b o)").rearrange("(p x) -> p x", x=1),
            )

            # --- raw images, fully contiguous loads: [128 parts, 128*128] ---
            raw = persist.tile([128, H, W], f32r)
            n_chunks = H // LOAD_CHUNK
            for b in range(BPC):
                p0 = 64 * b
                for k in range(n_chunks):
                    r0 = k * LOAD_CHUNK
                    nc.sync.dma_start(
                        out=raw[p0 : p0 + 64, r0 : r0 + LOAD_CHUNK, :],
                        in_=x_d[b, :, r0 : r0 + LOAD_CHUNK, :],
                    )

            # --- column-wrap padded image [128, 128, 130], built on-chip.
            # img[c, r, 0] = x[c, r, 127]; img[c, r, 1:129] = x[c, r, :];
            # img[c, r, 129] = x[c, r, 0]. Row wrap is NOT padded (handled by
            # split matmuls on boundary tiles). Spread copies across engines.
            img = persist.tile([128, H, WP], f32r)
            copy_engines = [nc.vector, nc.gpsimd, nc.scalar]

            def eng_copy(e, out, in_):
                if e is nc.scalar:
                    e.activation(
                        out=out, in_=in_, func=mybir.ActivationFunctionType.Copy
                    )
                else:
                    e.tensor_copy(out=out, in_=in_)

            for k in range(n_chunks):
                r0 = k * LOAD_CHUNK
                r1 = r0 + LOAD_CHUNK
                e = copy_engines[k % len(copy_engines)]
                eng_copy(e, img[:, r0:r1, 1 : 1 + W], raw[:, r0:r1, :])
                eng_copy(e, img[:, r0:r1, 0], img[:, r0:r1, W])
                eng_copy(e, img[:, r0:r1, WP - 1], img[:, r0:r1, 1])

            # --- main loop: 32 spatial tiles of 4 output rows.
            # Shift row order [1, 0, 2] so the first matmul of each tile is
            # always a full-coverage N=512 one (start=True zeroes the bank).
            def rhs_rows(i, h0):
                # image rows needed by kernel-row i for out rows h0..h0+3
                return h0 + i - 1

            for t in range(N_TILES):
                h0 = t * TILE_ROWS
                ps = psum.tile([128, TILE_ROWS, W], f32)
                mms = []  # (out_slice, rhs_ap)
                for i in (1, 0, 2):
                    for j in range(KW):
                        r = rhs_rows(i, h0)
                        lhsT = wts[:, i * KW + j, :]
                        if r < 0:
                            # t=0, i=0: out row 0 reads image row H-1
                            mms.append(
                                (lhsT, ps[:, 0:1, :], img[:, H - 1 : H, j : j + W])
                            )
                            mms.append(
                                (lhsT, ps[:, 1:TILE_ROWS, :],
                                 img[:, 0 : TILE_ROWS - 1, j : j + W])
                            )
                        elif r + TILE_ROWS > H:
                            # t=31, i=2: out row 3 reads image row 0
                            mms.append(
                                (lhsT, ps[:, 0 : TILE_ROWS - 1, :],
                                 img[:, r : H, j : j + W])
                            )
                            mms.append(
                                (lhsT, ps[:, TILE_ROWS - 1 : TILE_ROWS, :],
                                 img[:, 0:1, j : j + W])
                            )
                        else:
                            mms.append(
                                (lhsT, ps[:, :, :], img[:, r : r + TILE_ROWS, j : j + W])
                            )
                for n, (lhsT, out_sl, rhs) in enumerate(mms):
                    nc.tensor.matmul(
                        out_sl,
                        lhsT=lhsT,
                        rhs=rhs,
                        start=(n == 0),
                        stop=(n == len(mms) - 1),
                    )

                # bias merge into an 8-row staging tile; store once per pair
                if t % 2 == 0:
                    st = ostage.tile([128, 2 * TILE_ROWS, W], f32)
                half = (t % 2) * TILE_ROWS
                nc.scalar.activation(
                    out=st[:, half : half + TILE_ROWS, :],
                    in_=ps,
                    func=mybir.ActivationFunctionType.Identity,
                    bias=bias_sb,
                )
                if t % 2 == 1:
                    for b in range(BPC):
                        nc.sync.dma_start(
                            out=y_d[b, :, h0 - TILE_ROWS : h0 + TILE_ROWS, :],
                            in_=st[64 * b : 64 * b + 64],
                        )
    return nc


def _get_module():
    if "nc" not in _CACHE:
        nc = _build_module()
        # CoreSim can't run modules with post-inserted instructions, so the
        # wait split is applied only on the hardware path.
        _split_multi_waits(nc)
        _CACHE["nc"] = nc
    return _CACHE["nc"]


def _in_maps(input, weight, bias):
    zeros = np.zeros(KH * KW * 128, np.float32)
    maps = []
    for i in range(N_CORES):
        lo, hi = i * BPC, (i + 1) * BPC
        maps.append(
            {
                "input": np.ascontiguousarray(input[lo:hi]),
                "weight": np.ascontiguousarray(weight[lo:hi]),
                "bias": np.ascontiguousarray(bias[lo:hi]),
                "zeros": zeros,
            }
        )
    return maps


def kernel(input, weight, bias):
    from concourse.bass_utils import run_bass_kernel_spmd

    nc = _get_module()
    res = run_bass_kernel_spmd(
        nc, _in_maps(input, weight, bias), core_ids=list(range(N_CORES))
    )
    return np.concatenate([res.results[i]["out"] for i in range(N_CORES)], axis=0)


# revision 14
# speedup vs baseline: 1.1665x; 1.0083x over previous
"""Trainium2 Bass kernel for per-sample dynamic 3x3 conv (periodic padding).

y[b,o,h,w] = sum_{c,i,j} x[b,c,(h+i-1)%H,(w+j-1)%W] * wgt[b, c*9+i*3+j, o] + bias[b,o]

Shapes: x [16,64,128,128] f32, wgt [16,576,64] f32, bias [16,64] f32.

Sharding: data-parallel over batch, 2 samples per core on 8 cores.

Compute scheme: both per-core samples are packed into single 128x128
matmuls with block-diagonal stationary weights:
  lhsT[k,m] = W_s0[c,o] at (k=c, m=o), W_s1[c,o] at (k=64+c, m=64+o), else 0
  rhs[k,n]  = col-padded img_s0[c, pos] (k<64) / img_s1[c, pos] (k>=64)
so one matmul per 3x3 shift contracts C=64 for both samples at once
(full K=128, full M=128, N=512). Matmuls are float32r (1 cycle/row).

Data movement (descriptor-count bound on this part, so): images load
CONTIGUOUSLY into a raw SBUF tile (16KB runs); the column-wrap-padded
image [128, 128, 130] is built on-chip by DVE/ACT/GPSIMD copies; the
row wrap is handled by splitting the affected matmuls on the two
boundary spatial tiles. Output rows are staged 8 at a time so stores
are 4KB-contiguous-per-partition DMAs.
"""

import numpy as np

KH = KW = 3
B, C, O, H, W = 16, 64, 64, 128, 128
N_CORES = 8
BPC = B // N_CORES  # samples per core
WP = W + 2  # 130: column-wrap padded row length
TILE_ROWS = 4  # output rows per PSUM tile -> N = 4*128 = 512
N_TILES = H // TILE_ROWS
LOAD_CHUNK = 32  # image rows per interior load DMA / pad-build chunk
OGROUP = 8  # spatial tiles per output store group (32 rows)

_CACHE = {}


def _patch_tile_drain():
    """This container's walrus rejects Drain instructions carrying more than
    one sem wait (setupSyncWait: Too many sync wait commands). Re-emit the
    TileContext exit drain's waits as individual wait_ge instructions."""
    import concourse.tile as tile
    from concourse.vector_clock import ScopedClock

    if getattr(tile.TileContext, "_drain_patch_applied", False):
        return

    def _drain_and_barrier(self, tick_clock, wait_clock):
        nc = self.nc
        nop = nc.sync.nop(nofuse=True)
        wait_clock.add_sem_waits(nop.ins, ScopedClock({None: tick_clock.global_clock}))
        waits = list(nop.ins.sync_info.on_wait)
        nop.ins.sync_info.on_wait.clear()
        assert self.sems is not None
        by_name = {}
        for h in self.sems.allocated().values():
            by_name[getattr(h, "name", None)] = h
        for w in waits:
            h = by_name.get(w.ant_name)
            assert h is not None, f"no sem handle for {w.ant_name}"
            nc.sync.wait_ge(h, w.wait_value)
        nc.sync.drain()
        nc.all_engine_barrier()
        popped = nc._tile_sem_poison_stack.pop()
        assert popped is self._sem_poison
        nc.clear_and_free_semaphores(list(self.sems.allocated().values()))
        nc.all_engine_barrier()

    tile.TileContext._drain_and_barrier = _drain_and_barrier
    tile.TileContext._drain_patch_applied = True


def _split_multi_waits(nc, max_waits=1):
    """Same walrus limitation, general form: any instruction carrying more
    than one sem wait fails setupSyncWait. Hoist excess waits onto dedicated
    single-wait NOPs on the same engine, placed just before the instruction."""
    import concourse.mybir as mybir

    for f in nc.m.functions:
        for blk in f.blocks:
            out = []
            changed = False
            for inst in blk.instructions:
                si = getattr(inst, "sync_info", None)
                waits = list(si.on_wait) if si is not None else []
                if len(waits) > max_waits:
                    changed = True
                    for w in waits[:-max_waits]:
                        out.append(
                            mybir.InstNoOp(
                                name=nc.get_next_instruction_name(),
                                engine=inst.engine,
                                sync_info=mybir.SyncInfo(on_wait=[w], on_update=[]),
                                bass_nofuse=True,
                            )
                        )
                    si.on_wait.clear()
                    for w in waits[-max_waits:]:
                        si.on_wait.append(w)
                out.append(inst)
            if changed:
                blk.instructions = out


def _build_module():
    import concourse.bass as bass
    import concourse.mybir as mybir
    import concourse.tile as tile

    _patch_tile_drain()

    f32 = mybir.dt.float32
    f32r = mybir.dt.float32r

    nc = bass.Bass()
    # input/weight feed FP32r matmuls; the BIR verifier requires every
    # producer in that dataflow to be float32r-typed, so declare the whole
    # chain float32r. float32r is byte-identical to float32 host-side.
    x_d = nc.dram_tensor("input", [BPC, C, H, W], f32r, kind="ExternalInput")
    w_d = nc.dram_tensor("weight", [BPC, C * KH * KW, O], f32r, kind="ExternalInput")
    b_d = nc.dram_tensor("bias", [BPC, O], f32, kind="ExternalInput")
    # host-supplied zeros for the block-diag off-diagonal fill: memset can't
    # produce float32r (invalid ISA value type), a DMA can.
    z_d = nc.dram_tensor("zeros", [KH * KW * 128], f32r, kind="ExternalInput")
    y_d = nc.dram_tensor("out", [BPC, O, H, W], f32, kind="ExternalOutput")

    with tile.TileContext(nc) as tc:
        from contextlib import ExitStack

        ctx = ExitStack()
        with ctx:
            persist = ctx.enter_context(tc.tile_pool(name="persist", bufs=1))
            psum = ctx.enter_context(tc.tile_pool(name="psum", bufs=6, space="PSUM"))
            ostage = ctx.enter_context(tc.tile_pool(name="ostage", bufs=2))

            # --- weights: block-diag per shift, [128 parts, 9 shifts, 128].
            # Off-diag zero fill via broadcast DMA, then one DMA per sample:
            # per partition c the source rows c*9+s (s=0..8) are 9*64
            # contiguous floats.
            wts = persist.tile([128, KH * KW, 128], f32r)
            z_src = z_d.rearrange("(s o) -> s o", s=KH * KW)
            nc.sync.dma_start(
                out=wts,
                in_=bass.AP(
                    tensor=z_src.tensor,
                    offset=z_src.offset,
                    ap=[[0, 128], z_src.ap[0], z_src.ap[1]],
                ),
            )
            # contiguous staging load (64 descriptors/sample), then an
            # on-chip reshuffle into the strided block-diag layout — a direct
            # strided DMA would cost 576 descriptors per sample.
            wstage = persist.tile([128, KH * KW, O], f32r)
            for b in range(BPC):
                w_cso = w_d[b].rearrange("(c s) o -> c s o", s=KH * KW)  # [64,9,64]
                nc.sync.dma_start(
                    out=wstage[64 * b : 64 * b + 64, :, :], in_=w_cso
                )
                nc.vector.tensor_copy(
                    out=wts[64 * b : 64 * b + 64, :, 64 * b : 64 * b + 64],
                    in_=wstage[64 * b : 64 * b + 64, :, :],
                )

            # --- bias: [128, 1], s0 channels on parts 0-63, s1 on 64-127 ---
            bias_sb = persist.tile([128, 1], f32)
            nc.sync.dma_start(
                out=bias_sb,
                in_=b_d.rearrange("b o -> (b o)").rearrange("(p x) -> p x", x=1),
            )

            # --- raw images, fully contiguous loads: [128 parts, 128*128] ---
            raw = persist.tile([128, H, W], f32r)
            n_chunks = H // LOAD_CHUNK
            for b in range(BPC):
                p0 = 64 * b
                for k in range(n_chunks):
                    r0 = k * LOAD_CHUNK
                    nc.sync.dma_start(
                        out=raw[p0 : p0 + 64, r0 : r0 + LOAD_CHUNK, :],
                        in_=x_d[b, :, r0 : r0 + LOAD_CHUNK, :],
                    )

            # --- column-wrap padded image [128, 128, 130], built on-chip.
            # img[c, r, 0] = x[c, r, 127]; img[c, r, 1:129] = x[c, r, :];
            # img[c, r, 129] = x[c, r, 0]. Row wrap is NOT padded (handled by
            # split matmuls on boundary tiles). Spread copies across engines.
            img = persist.tile([128, H, WP], f32r)
            copy_engines = [nc.vector, nc.gpsimd, nc.scalar, nc.vector]

            def eng_copy(e, out, in_):
                if e is nc.scalar:
                    e.activation(
                        out=out, in_=in_, func=mybir.ActivationFunctionType.Copy
                    )
                else:
                    e.tensor_copy(out=out, in_=in_)

            for k in range(n_chunks):
                r0 = k * LOAD_CHUNK
                r1 = r0 + LOAD_CHUNK
                e = copy_engines[k % len(copy_engines)]
                eng_copy(e, img[:, r0:r1, 1 : 1 + W], raw[:, r0:r1, :])
                eng_copy(e, img[:, r0:r1, 0], img[:, r0:r1, W])
                eng_copy(e, img[:, r0:r1, WP - 1], img[:, r0:r1, 1])

            # --- main loop: 32 spatial tiles of 4 output rows.
            # Shift row order [1, 0, 2] so the first matmul of each tile is
            # always a full-coverage N=512 one (start=True zeroes the bank).
            def rhs_rows(i, h0):
                # image rows needed by kernel-row i for out rows h0..h0+3
                return h0 + i - 1

            for t in range(N_TILES):
                h0 = t * TILE_ROWS
                ps = psum.tile([128, TILE_ROWS, W], f32)
                mms = []  # (out_slice, rhs_ap)
                for i in (1, 0, 2):
                    for j in range(KW):
                        r = rhs_rows(i, h0)
                        lhsT = wts[:, i * KW + j, :]
                        if r < 0:
                            # t=0, i=0: out row 0 reads image row H-1
                            mms.append(
                                (lhsT, ps[:, 0:1, :], img[:, H - 1 : H, j : j + W])
                            )
                            mms.append(
                                (lhsT, ps[:, 1:TILE_ROWS, :],
                                 img[:, 0 : TILE_ROWS - 1, j : j + W])
                            )
                        elif r + TILE_ROWS > H:
                            # t=31, i=2: out row 3 reads image row 0
                            mms.append(
                                (lhsT, ps[:, 0 : TILE_ROWS - 1, :],
                                 img[:, r : H, j : j + W])
                            )
                            mms.append(
                                (lhsT, ps[:, TILE_ROWS - 1 : TILE_ROWS, :],
                                 img[:, 0:1, j : j + W])
                            )
                        else:
                            mms.append(
                                (lhsT, ps[:, :, :], img[:, r : r + TILE_ROWS, j : j + W])
                            )
                for n, (lhsT, out_sl, rhs) in enumerate(mms):
                    nc.tensor.matmul(
                        out_sl,
                        lhsT=lhsT,
                        rhs=rhs,
                        start=(n == 0),
                        stop=(n == len(mms) - 1),
                    )

                # bias merge into a 32-row staging tile (stores are then 8KB
                # contiguous per partition -> 64 descriptors per DMA).
                # Alternate ACT and DVE so neither engine serializes the PE.
                g = t % OGROUP
                if g == 0:
                    st = ostage.tile([128, OGROUP * TILE_ROWS, W], f32)
                row0 = g * TILE_ROWS
                if t % 2 == 0:
                    nc.scalar.activation(
                        out=st[:, row0 : row0 + TILE_ROWS, :],
                        in_=ps,
                        func=mybir.ActivationFunctionType.Identity,
                        bias=bias_sb,
                    )
                else:
                    nc.vector.tensor_scalar_add(
                        out=st[:, row0 : row0 + TILE_ROWS, :],
                        in0=ps,
                        scalar1=bias_sb,
                    )
                if g == OGROUP - 1:
                    g0 = (t - OGROUP + 1) * TILE_ROWS
                    for b in range(BPC):
                        nc.sync.dma_start(
                            out=y_d[b, :, g0 : g0 + OGROUP * TILE_ROWS, :],
                            in_=st[64 * b : 64 * b + 64],
                        )
    return nc


def _get_module():
    if "nc" not in _CACHE:
        nc = _build_module()
        # CoreSim can't run modules with post-inserted instructions, so the
        # wait split is applied only on the hardware path.
        _split_multi_waits(nc)
        _CACHE["nc"] = nc
    return _CACHE["nc"]


def _in_maps(input, weight, bias):
    zeros = np.zeros(KH * KW * 128, np.float32)
    maps = []
    for i in range(N_CORES):
        lo, hi = i * BPC, (i + 1) * BPC
        maps.append(
            {
                "input": np.ascontiguousarray(input[lo:hi]),
                "weight": np.ascontiguousarray(weight[lo:hi]),
                "bias": np.ascontiguousarray(bias[lo:hi]),
                "zeros": zeros,
            }
        )
    return maps


def kernel(input, weight, bias):
    from concourse.bass_utils import run_bass_kernel_spmd

    nc = _get_module()
    res = run_bass_kernel_spmd(
        nc, _in_maps(input, weight, bias), core_ids=list(range(N_CORES))
    )
    return np.concatenate([res.results[i]["out"] for i in range(N_CORES)], axis=0)
